# revision 48
# baseline (speedup 1.0000x reference)
"""Trainium2 Bass kernel for nn_NmpNet_batch (gnn_message_passing).

The reference network is linear everywhere except the two final ReLUs:
every edge feature is A[j] + C[i] + d (per-node terms), and the
edge->node aggregation sums those over i. So the N^2-edge graph
collapses algebraically to per-node matmuls plus per-scene column
biases; folding all chained linear layers (host-side, float64):

    hp   = [h | p]                                (N, 66) per scene
    y1   = relu(hp @ Whp1 + (Shp @ WhpX + bias_cx) @ f1_w + f1_b)
    out  = relu(y1 @ f2_w + f2_b)

where Shp = sum_nodes hp (per scene). The folded algebra is exact
(~2e-7 relative error in fp32); the shipped kernel runs the two big
matmuls in fp32r (full-rate PE mode), measured 1.5e-4 relative error
on hardware.

Device layout is feature-major ([feature, scene*node]); the host
pre-transposes the per-core input shard and un-transposes the output,
so the device does zero transposes: one input DMA, a per-scene
reduce feeding a tiny bias matmul, two half-width [*, 256] fp32r
matmuls per layer (fp32r needs a >=256 moving dim for full rate;
separate half tiles keep PSUM-bank deps decoupled), fused bias+relu
ops spread across DVE and ACT, and two output DMAs on different
rings. Data-parallel over the batch axis: 4 scenes/core on 8 cores,
params replicated. TimelineSim: ~9.9 us single-shot; HW-validated via
an in-NEFF For_i loop at ~10.5 us/iter including the ~2 us back-edge
barrier.
"""

import sys

import numpy as np

if "/opt/trn_rl_repo" not in sys.path:
    sys.path.insert(0, "/opt/trn_rl_repo")

B, N, HF = 32, 128, 64
NCORES = 8
S = B // NCORES  # scenes per core
SN = S * N       # nodes per core
HP = HF + 2      # h features + p features

# wallA column layout ([66, WA_C], f32) — gates the first matmul
A_WHP1 = 0            # [0:66, 0:128]    Whp1
A_WB1 = 128           # [0:66, 128:256]  Wb1 = WhpX @ f1_w
A_F2B = 256           # [0:64]   f2_b
WA_C = 257
# wallB column layout ([128, WB_C], f32) — needed a bit later
B_F2W = 0             # [0:128, 0:64]    f2_w
B_CB1 = 64            # [0:128]  cb1 = bias_cx @ f1_w + f1_b
WB_C = 65

_CACHE = {}
LAST_RESULTS = None  # BassKernelResults of the most recent run (for profiling)


def _fold_weights(i):
    """Fold every chained linear layer; float64 for accuracy."""
    f8 = lambda x: np.asarray(x, np.float64)
    se_w, se_b = f8(i["se_w"]), f8(i["se_b"])
    s1_w, s1_b = f8(i["s1_w"]), f8(i["s1_b"])
    s2_w, s2_b = f8(i["s2_w"]), f8(i["s2_b"])
    n1_w, n1_b = f8(i["n1_w"]), f8(i["n1_b"])
    n2_w, n2_b = f8(i["n2_w"]), f8(i["n2_b"])
    e1_w, e1_b = f8(i["e1_w"]), f8(i["e1_b"])
    e2_w, e2_b = f8(i["e2_w"]), f8(i["e2_b"])
    f1_w, f1_b = f8(i["f1_w"]), f8(i["f1_b"])
    f2_w, f2_b = f8(i["f2_w"]), f8(i["f2_b"])

    W_s = s1_w @ s2_w              # (192, 64)
    c_s = s1_b @ s2_w + s2_b       # (64,)
    Wr, Ws, We = W_s[0:64], W_s[64:128], W_s[128:192]
    Pse = se_w @ We                # (2, 64)
    d = se_b @ We + c_s            # (64,)
    W_n = n1_w @ n2_w
    c_n = n1_b @ n2_w + n2_b
    W_e = e1_w @ e2_w
    c_e = e1_b @ e2_w + e2_b
    Wer, Wes = W_e[0:64], W_e[64:128]

    # x = A @ W_nr + colX ⊗ 1,  A = h@Wr - p@Pse,  W_nr = 4*W_n@Wer
    # colX = Sh@WhX + Sp@WpX + bias_cx  (per-scene sums Sh, Sp)
    Wg = 2.0 * (Wer + Wes)
    Wsa = (W_n @ Wes) / 32.0
    Wngd = (W_n @ Wg) / 64.0
    WhX = Ws @ Wngd + Wr @ Wsa
    WpX = Pse @ (Wngd - Wsa)
    bias_cx = 2.0 * (d @ W_n @ Wg) + c_n @ Wg + 2.0 * c_e
    W_x1 = 4.0 * (W_n @ Wer) @ f1_w          # (64, 128)
    # y1 = relu(x @ f1_w + f1_b) and x -> y1 is linear, so fold through:
    Wh1 = Wr @ W_x1                          # (64, 128)
    Wp1 = -(Pse @ W_x1)                      # (2, 128)

    # b1 = colX @ f1_w + f1_b = Shp @ (WhpX @ f1_w) + (bias_cx @ f1_w + f1_b)
    Wb1 = np.vstack([WhX, WpX]) @ f1_w        # (66, 128)
    cb1 = bias_cx @ f1_w + f1_b               # (128,)

    wallA = np.zeros((HP, WA_C), np.float32)
    wallA[0:64, A_WHP1:A_WHP1 + 128] = Wh1
    wallA[64:66, A_WHP1:A_WHP1 + 128] = Wp1
    wallA[0:66, A_WB1:A_WB1 + 128] = Wb1
    wallA[0:64, A_F2B] = f2_b
    wallB = np.zeros((128, WB_C), np.float32)
    wallB[0:128, B_F2W:B_F2W + 64] = f2_w
    wallB[0:128, B_CB1] = cb1
    return wallA, wallB


def _build_nc(reps=1, relu_mix=True, use_f32r=True, one_b1=False,
              hp_swdge=False, ot_act=False, out_swdge=False, loop_k=None,
              y1_eng="ADAD", ot_eng="AA", hp_ksplit=False):
    # y1_eng: 4-char pattern over scenes, 'D'=DVE / 'A'=ACT (overrides
    # relu_mix); ot_eng: 2-char pattern over halves (overrides ot_act).
    import contextlib

    import concourse.bacc as bacc
    import concourse.tile as tile
    from concourse import mybir

    f32 = mybir.dt.float32
    f32r = mybir.dt.float32r if use_f32r else mybir.dt.float32
    AX = mybir.AxisListType
    ALU = mybir.AluOpType
    ACTF = mybir.ActivationFunctionType

    nc = bacc.Bacc()
    wa_d = nc.dram_tensor("wallA", [HP, WA_C], f32r, kind="ExternalInput")
    wb_d = nc.dram_tensor("wallB", [128, WB_C], f32r, kind="ExternalInput")
    hp_d = nc.dram_tensor("hp", [HP, SN], f32r, kind="ExternalInput")
    out_d = nc.dram_tensor("out", [HF, SN], f32, kind="ExternalOutput")

    with tile.TileContext(nc) as tc:
        with (
            tc.tile_pool(name="const", bufs=1) as cpool,
            tc.tile_pool(name="sb", bufs=2) as spool,
            tc.tile_pool(name="ps", bufs=3, space="PSUM") as ppool,
            tc.tile_pool(name="pss", bufs=2, space="PSUM") as qpool,
        ):
            # wallA (tiny, gates the first matmul) rides the Activation
            # sequencer; wallB rides PE's — so neither serializes behind
            # hp's issue on SP.
            wa = cpool.tile([HP, WA_C], f32r)
            nc.scalar.dma_start(wa, wa_d[:, :])
            wb = cpool.tile([128, WB_C], f32r)
            if hp_swdge:
                nc.sync.dma_start(wb, wb_d[:, :])
            else:
                nc.gpsimd.dma_start(wb, wb_d[:, :])

            loop_cm = (tc.For_i(0, loop_k, 1) if loop_k is not None
                       else contextlib.nullcontext())
            with loop_cm:
              for _rep in range(reps):
                hp = spool.tile([HP, SN], f32r, tag="hp")
                if hp_ksplit:
                    # Row-split: half the descriptors per DMA, so the first
                    # K-half lands ~0.5us earlier and the matmuls accumulate
                    # over the two K-halves as they arrive.
                    KH = 32
                    nc.sync.dma_start(hp[0:KH, :], hp_d[0:KH, :])
                    nc.sync.dma_start(hp[KH:HP, :], hp_d[KH:HP, :])
                elif hp_swdge:
                    nc.gpsimd.dma_start(hp, hp_d[:, :])
                else:
                    nc.sync.dma_start(hp, hp_d[:, :])

                # Per-scene node sums Shp[:, s] = sum_n hp[:, s*N+n].
                # Scene 0 is reduced first so b1[:, 0] unblocks the first
                # relu as early as possible; the same PSUM bank naturally
                # serializes mm0 -> add0 -> mm123 -> add123.
                # The b1 matmuls are tiny (1-3 moving columns) — fp32r ISA
                # restrictions disallow them, so this path stays plain f32.
                Shp = spool.tile([HP, S], f32, tag="Shp")
                ps_b1 = qpool.tile([128, S], f32, tag="small", bufs=1)
                b1 = spool.tile([128, S], f32, tag="b1")
                wab1 = wa[0:HP, A_WB1:A_WB1 + 128].bitcast(f32)
                cb1ap = wb[:, B_CB1:B_CB1 + 1].bitcast(f32)
                if hp_ksplit:
                    KH = 32
                    nc.vector.reduce_sum(
                        Shp[0:KH, :],
                        hp[0:KH, :].bitcast(f32).rearrange(
                            "p (s n) -> p s n", s=S),
                        axis=AX.X)
                    nc.vector.reduce_sum(
                        Shp[KH:HP, :],
                        hp[KH:HP, :].bitcast(f32).rearrange(
                            "p (s n) -> p s n", s=S),
                        axis=AX.X)
                    nc.tensor.matmul(ps_b1,
                                     wa[0:KH, A_WB1:A_WB1 + 128].bitcast(f32),
                                     Shp[0:KH, :], start=True, stop=False)
                    nc.tensor.matmul(ps_b1,
                                     wa[KH:HP, A_WB1:A_WB1 + 128].bitcast(f32),
                                     Shp[KH:HP, :], start=False, stop=True)
                    nc.vector.tensor_scalar_add(b1, ps_b1, cb1ap)
                elif one_b1:
                    nc.vector.reduce_sum(
                        Shp, hp.bitcast(f32).rearrange("p (s n) -> p s n", s=S),
                        axis=AX.X)
                    nc.tensor.matmul(ps_b1, wab1, Shp, start=True, stop=True)
                    nc.vector.tensor_scalar_add(b1, ps_b1, cb1ap)
                else:
                    nc.vector.reduce_sum(
                        Shp[:, 0:1],
                        hp[:, 0:N].bitcast(f32).rearrange("p (s n) -> p s n",
                                                          s=1),
                        axis=AX.X)
                    nc.tensor.matmul(ps_b1[:, 0:1], wab1, Shp[:, 0:1],
                                     start=True, stop=True)
                    nc.vector.tensor_scalar_add(b1[:, 0:1], ps_b1[:, 0:1],
                                                cb1ap)
                    nc.vector.reduce_sum(
                        Shp[:, 1:S],
                        hp[:, N:SN].bitcast(f32).rearrange("p (s n) -> p s n",
                                                           s=S - 1),
                        axis=AX.X)
                    nc.tensor.matmul(ps_b1[:, 1:S], wab1, Shp[:, 1:S],
                                     start=True, stop=True)
                    nc.vector.tensor_scalar_add(b1[:, 1:S], ps_b1[:, 1:S],
                                                cb1ap)

                # y1 = relu(Whp1.T @ hp + b1), then out = relu(f2_w.T @ y1
                # + f2_b). Two halves (2 scenes each): fp32r runs at full
                # PE rate only when the moving free dim is >= 256, and
                # separate half tiles keep PSUM-bank/tile deps decoupled so
                # half 0 flows while half 1 computes.
                H2 = SN // 2  # columns per half
                y1h, oth = [], []
                for hx in range(2):
                    ps_y1 = ppool.tile([128, H2], f32, tag="py1", bufs=2,
                                       name=f"py1_{hx}")
                    if hp_ksplit:
                        KH = 32
                        nc.tensor.matmul(
                            ps_y1, wa[0:KH, A_WHP1:A_WHP1 + 128],
                            hp[0:KH, hx * H2:(hx + 1) * H2],
                            start=True, stop=False)
                        nc.tensor.matmul(
                            ps_y1, wa[KH:HP, A_WHP1:A_WHP1 + 128],
                            hp[KH:HP, hx * H2:(hx + 1) * H2],
                            start=False, stop=True)
                    else:
                        nc.tensor.matmul(
                            ps_y1, wa[0:HP, A_WHP1:A_WHP1 + 128],
                            hp[:, hx * H2:(hx + 1) * H2],
                            start=True, stop=True)
                    y1 = spool.tile([128, H2], f32r, tag=f"y1_{hx}",
                                    name=f"y1_{hx}")
                    for sx in range(2):
                        s = hx * 2 + sx
                        on_act = (y1_eng[s] == "A") if y1_eng else \
                            (relu_mix and sx == 1)
                        if on_act:
                            nc.scalar.activation(
                                y1[:, sx * N:(sx + 1) * N],
                                ps_y1[:, sx * N:(sx + 1) * N],
                                ACTF.Relu, bias=b1[:, s:s + 1], scale=1.0)
                        else:
                            nc.vector.tensor_scalar(
                                out=y1[:, sx * N:(sx + 1) * N],
                                in0=ps_y1[:, sx * N:(sx + 1) * N],
                                scalar1=b1[:, s:s + 1], scalar2=0.0,
                                op0=ALU.add, op1=ALU.max)
                    ps_y2 = ppool.tile([64, H2], f32, tag="py2", bufs=2,
                                       name=f"py2_{hx}")
                    nc.tensor.matmul(ps_y2, wb[:, B_F2W:B_F2W + 64], y1,
                                     start=True, stop=True)
                    oT = spool.tile([64, H2], f32, tag=f"oT_{hx}",
                                    name=f"oT_{hx}")
                    ot_on_act = (ot_eng[hx] == "A") if ot_eng else \
                        (ot_act and hx == 0)
                    if ot_on_act:
                        nc.scalar.activation(
                            oT, ps_y2, ACTF.Relu,
                            bias=wa[0:64, A_F2B:A_F2B + 1].bitcast(f32),
                            scale=1.0)
                    else:
                        nc.vector.tensor_scalar(
                            out=oT, in0=ps_y2,
                            scalar1=wa[0:64, A_F2B:A_F2B + 1].bitcast(f32),
                            scalar2=0.0, op0=ALU.add, op1=ALU.max)
                    y1h.append(y1)
                    oth.append(oT)

                # Two output DMAs on different rings so the first half ships
                # while the second is still computing.
                nc.sync.dma_start(out_d[:, 0:H2], oth[0])
                if out_swdge:
                    nc.gpsimd.dma_start(out_d[:, H2:SN], oth[1])
                else:
                    nc.scalar.dma_start(out_d[:, H2:SN], oth[1])

    nc.finalize()
    return nc


def _prep_in_maps(h, p, wallA, wallB):
    in_maps = []
    for c in range(NCORES):
        hpT = np.empty((HP, SN), np.float32)
        hpT[0:HF] = h[c * S:(c + 1) * S].transpose(2, 0, 1).reshape(HF, SN)
        hpT[HF:HP] = p[c * S:(c + 1) * S].transpose(2, 0, 1).reshape(2, SN)
        in_maps.append({"hp": hpT, "wallA": wallA, "wallB": wallB})
    return in_maps


def kernel(**inputs):
    from concourse.bass_utils import run_bass_kernel_spmd

    global LAST_RESULTS

    h = np.asarray(inputs["h_states"], np.float32)
    p = np.asarray(inputs["end_pos"], np.float32)
    wallA, wallB = _fold_weights(inputs)

    if "nc" not in _CACHE:
        _CACHE["nc"] = _build_nc()
    nc = _CACHE["nc"]

    in_maps = _prep_in_maps(h, p, wallA, wallB)
    res = run_bass_kernel_spmd(nc, in_maps, core_ids=list(range(NCORES)))
    LAST_RESULTS = res
    # oT per core is [64, S*128] feature-major -> back to (S, N, HF)
    out = np.concatenate(
        [r["out"].reshape(HF, S, N).transpose(1, 2, 0) for r in res.results],
        axis=0)
    return np.ascontiguousarray(out.astype(np.float32))
